# revision 5
# baseline (speedup 1.0000x reference)
"""Tensor-parallel fused attention kernel for Trainium2 (8 NeuronCores).

Problem: x[2,2048,4096] -> QKV proj (GQA 32q/8kv heads, head_dim 128) ->
RoPE -> causal attention -> out proj, all f32 I/O.

Sharding: tensor-parallel over heads. Core c gets q heads 4c..4c+3 and
kv head c (w_qkv rows), plus the matching 512 columns of w_o. x is
replicated (transposed + bf16-cast on host). Each core emits a partial
y [4096, 4096]; the host sums the 8 partials.

On-chip compute is bf16 matmuls with fp32 PSUM accumulation; softmax is
exp in fp32 (scores bounded ~|5.6| for this input distribution, so no
max-subtraction is needed) with fp32 denominators.

v2 layout strategy:
- Phase 1 (QKV proj): x stationary, 768 moving qkv columns per
  LDWEIGHTS; RoPE in token-natural layout; q/k flipped into [head_dim,
  token] attention layout via the DMA xbar transpose (off the PE).
- Phase 2+3 fused: per 512-query block, scores^T = k_chunk.T @ q with
  causal column-range restriction on diagonal chunks; exp on ScalarE;
  denominator accumulated by DVE adds; normalization = one all-ones
  matmul (column-sum + broadcast in one op) + reciprocal_approx_fast +
  one DVE multiply. PV runs one chunk behind scores (software
  pipelining) and out-projection matmul groups for completed blocks
  drain from a FIFO between attention slots so the PE never starves on
  the exp latency.
"""

import numpy as np
import ml_dtypes

import concourse.bass as bass
import concourse.mybir as mybir
import concourse.tile as tile
from concourse import bacc
from concourse.bass_utils import run_bass_kernel_spmd

F32 = mybir.dt.float32
BF16 = mybir.dt.bfloat16
AF = mybir.ActivationFunctionType
BF = ml_dtypes.bfloat16

# Model dims (hardcoded per contract)
B, S, D = 2, 2048, 4096
H, KV, DH = 32, 8, 128
T = B * S                     # 4096 tokens, batch-major
N_CORES = 8
HPC = H // N_CORES            # 4 q heads per core
QKV_ROWS = HPC * DH + 2 * DH  # 768 rows of w_qkv per core
WO_COLS = HPC * DH            # 512 w_o columns per core
SCALE = 1.0 / np.sqrt(DH)

KCH = D // 128                # 32 contraction chunks
SQ = 512                      # phase-2 q block
N_QB = S // SQ                # 4 q blocks per sequence


def _build_nc():
    nc = bacc.Bacc()

    xP = nc.declare_dram_parameter("xP", [T // 256, 128, KCH * 256], BF16,
                                   isOutput=False)
    wqT = nc.declare_dram_parameter("wqT", [D, QKV_ROWS], BF16, isOutput=False)
    woT = nc.declare_dram_parameter("woT", [WO_COLS, D], BF16, isOutput=False)
    cs4 = nc.declare_dram_parameter("cs4", [128, S // 128 * 256], BF16,
                                    isOutput=False)
    sn4 = nc.declare_dram_parameter("sn4", [128, S // 128 * 256], BF16,
                                    isOutput=False)
    maskT = nc.declare_dram_parameter("maskT", [128, 128], BF16,
                                      isOutput=False)
    y = nc.declare_dram_parameter("y", [T, D], BF16, isOutput=True)

    wqT3 = wqT.rearrange("(ko p) m -> p ko m", p=128)   # [128, 32, 768]
    woT3 = woT.rearrange("(h p) d -> p h d", p=128)     # [128, 4, 4096]
    y3 = y.rearrange("(tm p) d -> p tm d", p=128)       # [128, 32, 4096]

    with tile.TileContext(nc) as tc:
        with tc.tile_pool(name="persist", bufs=1) as persist:

            # --- persistent tiles ---
            maskT_t = persist.tile([128, 128], BF16)
            nc.sync.dma_start(maskT_t[:], maskT[:])
            ones_bf = persist.tile([128, 128], BF16)
            nc.vector.memset(ones_bf[:], 1.0)
            # wo is DMA'd later (inside phase 1) so the startup queue serves
            # wq + the first x tiles first; first use is deep into phase 2.
            wo = persist.tile([128, HPC, D], BF16)

            # attention-layout q/k storage [DH, T]; v natural [tok, DH]
            qkT = [persist.tile([128, T], BF16, tag=f"qk{m}", name=f"qk{m}")
                   for m in range(5)]
            v_nat = persist.tile([128, T // 128, 128], BF16)

            # ============ Phase 1: QKV projection (x-stationary) + RoPE ======
            with tc.tile_pool(name="p1", bufs=3) as p1, \
                 tc.tile_pool(name="p1w", bufs=1) as p1w, \
                 tc.tile_pool(name="p1s", bufs=3) as p1s, \
                 tc.tile_pool(name="psQ", bufs=2, space="PSUM") as psQ, \
                 tc.tile_pool(name="psV2", bufs=2, space="PSUM") as psV2:
                wq = p1w.tile([128, KCH, QKV_ROWS], BF16, tag="wq")
                for ko in range(KCH):
                    nc.sync.dma_start(wq[:, ko, :], wqT3[:, ko, :])
                cs4_t = p1w.tile([128, S // 128, 4, 64], BF16, tag="cs4")
                sn4_t = p1w.tile([128, S // 128, 4, 64], BF16, tag="sn4")
                nc.sync.dma_start(
                    cs4_t[:].rearrange("p g h j -> p (g h j)"), cs4[:])
                nc.sync.dma_start(
                    sn4_t[:].rearrange("p g h j -> p (g h j)"), sn4[:])

                for gg in range(T // 256):       # 256-token load granularity
                    xt = p1.tile([128, KCH, 256], BF16, tag="xt")
                    nc.sync.dma_start(
                        xt[:].rearrange("p a b -> p (a b)"), xP[gg])
                    if gg == 2:
                        for h in range(HPC):
                            nc.sync.dma_start(wo[:, h, :], woT3[:, h, :])
                    for half in range(2):
                        g = gg * 2 + half        # 128-token chunk index
                        pq = psQ.tile([128, 512], F32, tag="pq")
                        pv2 = psV2.tile([128, 256], F32, tag="pv2")
                        for k in range(KCH):
                            lhs = xt[:, k, half * 128:(half + 1) * 128]
                            nc.tensor.matmul(pq[:], lhs, wq[:, k, 0:512],
                                             start=(k == 0), stop=(k == KCH - 1))
                            nc.tensor.matmul(pv2[:], lhs, wq[:, k, 512:768],
                                             start=(k == 0), stop=(k == KCH - 1))
                        # copy to bf16 natural staging
                        pre = p1s.tile([128, QKV_ROWS], BF16, tag="pre")
                        nc.scalar.copy(pre[:, 0:512], pq[:])
                        nc.scalar.copy(pre[:, 512:768], pv2[:])
                        # v: straight to v_nat
                        nc.vector.tensor_copy(v_nat[:, g, :], pre[:, 640:768])
                        csg = cs4_t[:, g % 16]   # [128, 4, 64] view (batch-periodic)
                        sng = sn4_t[:, g % 16]
                        nat = p1s.tile([128, 640], BF16, tag="nat")
                        q4 = pre[:, 0:512].rearrange("p (h two j) -> p h two j",
                                                     two=2, j=64)
                        n4 = nat[:, 0:512].rearrange("p (h two j) -> p h two j",
                                                     two=2, j=64)
                        tA = p1s.tile([128, 4, 64], BF16, tag="tA")
                        tB = p1s.tile([128, 4, 64], BF16, tag="tB")
                        # q rope (4 heads batched)
                        nc.vector.tensor_mul(tA[:], q4[:, :, 0, :], csg)
                        nc.vector.tensor_mul(tB[:], q4[:, :, 1, :], sng)
                        nc.vector.tensor_sub(n4[:, :, 0, :], tA[:], tB[:])
                        nc.vector.tensor_mul(tA[:], q4[:, :, 1, :], csg)
                        nc.vector.tensor_mul(tB[:], q4[:, :, 0, :], sng)
                        nc.vector.tensor_add(n4[:, :, 1, :], tA[:], tB[:])
                        # k rope
                        nc.vector.tensor_mul(tA[:, 0, :], pre[:, 512:576], csg[:, 0, :])
                        nc.vector.tensor_mul(tB[:, 0, :], pre[:, 576:640], sng[:, 0, :])
                        nc.vector.tensor_sub(nat[:, 512:576], tA[:, 0, :], tB[:, 0, :])
                        nc.vector.tensor_mul(tA[:, 0, :], pre[:, 576:640], csg[:, 0, :])
                        nc.vector.tensor_mul(tB[:, 0, :], pre[:, 512:576], sng[:, 0, :])
                        nc.vector.tensor_add(nat[:, 576:640], tA[:, 0, :], tB[:, 0, :])
                        # flip q0..q3,k into attention layout via the DMA xbar
                        for m in range(5):
                            nc.sync.dma_start_transpose(
                                qkT[m][:, g * 128:(g + 1) * 128],
                                nat[:, m * 128:(m + 1) * 128])

            # ============ Phase 2+3 fused: attention + out-projection ========
            with tc.tile_pool(name="p2", bufs=3) as p2, \
                 tc.tile_pool(name="p2acc", bufs=2) as p2acc, \
                 tc.tile_pool(name="p2n", bufs=2) as p2n, \
                 tc.tile_pool(name="p2o", bufs=3) as p2o, \
                 tc.tile_pool(name="p2y", bufs=4) as p2y, \
                 tc.tile_pool(name="psS", bufs=3, space="PSUM") as psS, \
                 tc.tile_pool(name="psO", bufs=1, space="PSUM") as psO, \
                 tc.tile_pool(name="psN", bufs=1, space="PSUM") as psN, \
                 tc.tile_pool(name="psY", bufs=2, space="PSUM") as psY:

                k_t = qkT[4]
                # FIFO of out-projection groups from completed blocks:
                # (outT_tile, tmg, dn, parity)
                pending = []

                def emit_outproj(n):
                    for _ in range(min(n, len(pending))):
                        outT_tile, tmg, dn, parity = pending.pop(0)
                        py = psY.tile([128, 512], F32, tag="py")
                        for hh in range(HPC):
                            nc.tensor.matmul(
                                py[:], outT_tile[:, hh, (tmg % 4) * 128:
                                                 (tmg % 4 + 1) * 128],
                                wo[:, hh, dn * 512:(dn + 1) * 512],
                                start=(hh == 0), stop=(hh == HPC - 1))
                        ysb = p2y.tile([128, 512], BF16, tag="ysb")
                        if parity:
                            nc.scalar.copy(ysb[:], py[:])
                        else:
                            nc.vector.tensor_copy(ysb[:], py[:])
                        nc.sync.dma_start(
                            y3[:, tmg, dn * 512:(dn + 1) * 512], ysb[:])

                for b in range(B):
                    tb = b * S
                    for qb in range(N_QB):
                        q0 = tb + qb * SQ
                        nki = 4 * qb + 4
                        outT_blk = p2o.tile([128, HPC, SQ], BF16, tag="outT")
                        for hg in range(2):
                            heads = (2 * hg, 2 * hg + 1)
                            accs = {h: p2acc.tile([128, SQ], F32,
                                                  tag=f"acc{h % 2}",
                                                  name=f"acc{h}")
                                    for h in heads}
                            pos = {h: psO.tile([128, SQ], F32,
                                               tag=f"po{h % 2}",
                                               name=f"po{h}") for h in heads}
                            prev = None
                            for ki in range(nki):
                                dj = ki - 4 * qb
                                off = 128 * dj if dj > 0 else 0
                                ksl = k_t[:, tb + ki * 128:
                                          tb + (ki + 1) * 128]
                                prs = {}
                                for h in heads:
                                    pss = psS.tile([128, SQ], F32, tag="ss")
                                    nc.tensor.matmul(
                                        pss[:, off:], ksl,
                                        qkT[h][:, q0 + off:q0 + SQ],
                                        start=True, stop=True)
                                    pr = p2.tile([128, SQ], BF16,
                                                 tag=f"pr{h % 2}")
                                    nc.scalar.activation(
                                        pr[:, off:], pss[:, off:], AF.Exp,
                                        scale=SCALE)
                                    if dj >= 0:
                                        nc.vector.tensor_mul(
                                            pr[:, off:off + 128],
                                            pr[:, off:off + 128], maskT_t[:])
                                    if ki == 0:
                                        nc.vector.tensor_copy(accs[h][:],
                                                              pr[:])
                                    else:
                                        nc.vector.tensor_add(
                                            accs[h][:, off:],
                                            accs[h][:, off:], pr[:, off:])
                                    prs[h] = (pr, off)
                                # software-pipelined PV: previous chunk
                                if prev is not None:
                                    pki, pprs = prev
                                    vsl = v_nat[:, (tb // 128) + pki, :]
                                    for h in heads:
                                        ppr, poff = pprs[h]
                                        nc.tensor.matmul(
                                            pos[h][:, poff:], vsl,
                                            ppr[:, poff:],
                                            start=(pki == 0), stop=False)
                                prev = (ki, prs)
                                emit_outproj(2)
                            # drain last PV
                            pki, pprs = prev
                            vsl = v_nat[:, (tb // 128) + pki, :]
                            for h in heads:
                                ppr, poff = pprs[h]
                                nc.tensor.matmul(pos[h][:, poff:], vsl,
                                                 ppr[:, poff:],
                                                 start=(pki == 0), stop=True)
                            emit_outproj(2)
                            # normalize: colsum+broadcast via ones-matmul,
                            # reciprocal, one multiply
                            for h in heads:
                                accb = p2n.tile([128, SQ], BF16, tag="accb")
                                nc.vector.tensor_copy(accb[:], accs[h][:])
                                denB = psN.tile([128, SQ], F32, tag="denB")
                                nc.tensor.matmul(denB[:], ones_bf[:],
                                                 accb[:], start=True,
                                                 stop=True)
                                recS = p2n.tile([128, SQ], F32, tag="recS")
                                nc.vector.reciprocal_approx_fast(
                                    recS[:], denB[:])
                                nc.vector.tensor_mul(outT_blk[:, h, :],
                                                     pos[h][:], recS[:])
                        # queue this block's out-projection
                        tmg0 = (tb + qb * SQ) // 128
                        for t in range(4):
                            for dn in range(D // 512):
                                pending.append((outT_blk, tmg0 + t, dn,
                                                (t * 8 + dn) % 2))
                emit_outproj(len(pending))

    nc.finalize()
    return nc


_NC_CACHE = None


def _get_nc():
    global _NC_CACHE
    if _NC_CACHE is None:
        _NC_CACHE = _build_nc()
    return _NC_CACHE


def _host_tables():
    inv_freq = 1.0 / (500000.0 ** (np.arange(0, DH, 2, dtype=np.float32) / DH))
    # token-natural tables: cs[p, g, j] = cos(pos(g*128+p) * inv_freq[j]),
    # duplicated 4x along a head axis for the batched q rope
    pos = (np.arange(T) % S).astype(np.float32)          # [T]
    fr = pos[:, None] * inv_freq[None, :]                # [T, 64]
    cos = np.cos(fr).astype(np.float32)
    sin = np.sin(fr).astype(np.float32)
    csn = cos.reshape(T // 128, 128, 64).transpose(1, 0, 2)[:, :S // 128]
    snn = sin.reshape(T // 128, 128, 64).transpose(1, 0, 2)[:, :S // 128]
    cs4 = np.broadcast_to(csn[:, :, None, :], (128, S // 128, 4, 64))
    sn4 = np.broadcast_to(snn[:, :, None, :], (128, S // 128, 4, 64))
    cs4 = np.ascontiguousarray(cs4).reshape(128, -1)
    sn4 = np.ascontiguousarray(sn4).reshape(128, -1)
    # triangular mask for the diagonal 128x128 block: valid iff j >= p
    j = np.arange(128)[None, :]
    p = np.arange(128)[:, None]
    m = (j >= p)
    return cs4.astype(BF), sn4.astype(BF), m.astype(BF)


def kernel(x: np.ndarray, w_qkv: np.ndarray, w_o: np.ndarray) -> np.ndarray:
    x = np.asarray(x, np.float32)
    w_qkv = np.asarray(w_qkv, np.float32)
    w_o = np.asarray(w_o, np.float32)
    nc = _get_nc()
    cs4, sn4, maskT = _host_tables()

    xTf = x.reshape(T, D).T.astype(BF)                           # [D, T]
    # pack: xP[gg, p, ko*256 + t] = xT[ko*128 + p, gg*256 + t]
    xP = np.ascontiguousarray(
        xTf.reshape(KCH, 128, T // 256, 256).transpose(2, 1, 0, 3)
           .reshape(T // 256, 128, KCH * 256))
    in_maps = []
    for c in range(N_CORES):
        rows = np.concatenate([
            np.arange(4 * c * DH, (4 * c + 4) * DH),             # 4 q heads
            np.arange(H * DH + c * DH, H * DH + (c + 1) * DH),   # k head
            np.arange((H + KV) * DH + c * DH, (H + KV) * DH + (c + 1) * DH),  # v head
        ])
        wqT = np.ascontiguousarray(w_qkv[rows, :].T).astype(BF)  # [D, 768]
        woT = np.ascontiguousarray(
            w_o[:, c * WO_COLS:(c + 1) * WO_COLS].T).astype(BF)  # [512, D]
        in_maps.append({
            "xP": xP, "wqT": wqT, "woT": woT,
            "cs4": cs4, "sn4": sn4, "maskT": maskT,
        })

    res = run_bass_kernel_spmd(nc, in_maps, core_ids=list(range(N_CORES)))
    globals()['_LAST_RESULT'] = res
    out = np.zeros((T, D), np.float32)
    for c in range(N_CORES):
        out += res.results[c]["y"].astype(np.float32)
    return out.reshape(B, S, D)
